# revision 1
# baseline (speedup 1.0000x reference)
"""LBP (local binary pattern) extractor on 8 Trainium2 NeuronCores.

Reference semantics (for each pixel p and its 8 neighbors n_k in clockwise
order with weights 1,2,4,...,128):
    bit_k = (img[p + off_k] >= img[p]),  where index -1 wraps (python
    negative indexing) and index >= size contributes 0.
    out = sum_k w_k * bit_k   (uint8)

Strategy:
  * Shard rows across 8 cores (1024 rows each) - embarrassingly parallel.
  * Host builds a padded slab per core: +1 halo row top/bottom and +1 halo
    col left/right.  Low-edge halos carry the wrapped row/col (python -1
    indexing); high-edge halos carry a -3e38 sentinel so `neighbor >= center`
    is identically False (the reference's IndexError -> bit 0 case).  This
    makes the device kernel completely uniform - no edge special-casing.
  * Device kernel per tile ([128 out rows] x [CW cols]):
      - DMA three row-shifted fp32 copies (up/center/down) into SBUF, so
        every engine access pattern starts at partition 0 (HW constraint:
        engine SBUF APs may only start at partitions 0/32/64/96).
      - 8x DVE tensor_tensor(is_ge) with column-shifted access patterns
        -> 8 bf16 0/1 bitplanes.
      - PE merges the 8 planes with weighted-identity matmuls accumulating
        in PSUM (weights 2^k on the diagonals) - byte assembly is free.
      - ACT copies PSUM -> uint8 SBUF, DMA out.
"""

import numpy as np

H = 8192
W = 8192
NCORES = 8
RPC = H // NCORES  # rows per core

CW = 2048  # columns per tile
TR = 128  # output rows per row tile
MMW = 512  # matmul moving free dim

# (dx, dy, weight) in the reference's clockwise order
OFFSETS = [
    (-1, -1, 1), (-1, 0, 2), (-1, 1, 4), (0, 1, 8),
    (1, 1, 16), (1, 0, 32), (1, -1, 64), (0, -1, 128),
]

SENTINEL = -3.0e38  # < any finite image value


def _build_bass():
    import concourse.bacc as bacc
    import concourse.mybir as mybir
    from concourse.tile import TileContext

    f32 = mybir.dt.float32
    bf16 = mybir.dt.bfloat16
    u8 = mybir.dt.uint8

    nc = bacc.Bacc("TRN2", target_bir_lowering=False)
    x = nc.dram_tensor("x", [RPC + 2, W + 2], f32, kind="ExternalInput")
    wident = nc.dram_tensor("wident", [128, 8 * 128], bf16, kind="ExternalInput")
    y = nc.dram_tensor("y", [RPC, W], u8, kind="ExternalOutput")

    n_row_tiles = (RPC + TR - 1) // TR
    n_col_chunks = W // CW

    with TileContext(nc) as tc:
        with (
            tc.tile_pool(name="const", bufs=1) as cpool,
            tc.tile_pool(name="img", bufs=2) as ipool,
            tc.tile_pool(name="planes", bufs=2) as ppool,
            tc.tile_pool(name="outb", bufs=3) as opool,
            tc.tile_pool(name="psum", bufs=8, space="PSUM") as qpool,
        ):
            wid = cpool.tile([128, 8 * 128], bf16)
            nc.sync.dma_start(wid[:, :], wident[:, :])

            for rt in range(n_row_tiles):
                r0 = rt * TR
                tr = min(TR, RPC - r0)
                for ct in range(n_col_chunks):
                    c0 = ct * CW
                    # img_s[d][p, :] = padded slab row (r0 + p + d), i.e.
                    # image row (r0 + p + d - 1): d=0 up, d=1 center, d=2 down
                    img_s = []
                    for d in range(3):
                        t = ipool.tile([128, CW + 2], f32, tag=f"img{d}")
                        nc.sync.dma_start(
                            t[0:tr, :], x[r0 + d : r0 + d + tr, c0 : c0 + CW + 2]
                        )
                        img_s.append(t)
                    ctr = img_s[1]
                    planes = []
                    for dx, dy, _w in OFFSETS:
                        pl = ppool.tile([128, CW], bf16, tag=f"pl{dx}{dy}")
                        nc.vector.tensor_tensor(
                            out=pl[0:tr, :],
                            in0=img_s[1 + dx][0:tr, 1 + dy : 1 + dy + CW],
                            in1=ctr[0:tr, 1 : 1 + CW],
                            op=mybir.AluOpType.is_ge,
                        )
                        planes.append(pl)
                    ou = opool.tile([128, CW], u8, tag="out")
                    for q in range(CW // MMW):
                        ps = qpool.tile([128, MMW], f32, tag="ps")
                        for k in range(8):
                            nc.tensor.matmul(
                                ps[0:tr, :],
                                lhsT=wid[0:tr, 128 * k : 128 * k + tr],
                                rhs=planes[k][0:tr, q * MMW : (q + 1) * MMW],
                                start=(k == 0),
                                stop=(k == 7),
                            )
                        nc.scalar.copy(
                            ou[0:tr, q * MMW : (q + 1) * MMW], ps[0:tr, :]
                        )
                    nc.sync.dma_start(y[r0 : r0 + tr, c0 : c0 + CW], ou[0:tr, :])

    nc.compile()
    return nc


_NC_CACHE = None


def _get_nc():
    global _NC_CACHE
    if _NC_CACHE is None:
        _NC_CACHE = _build_bass()
    return _NC_CACHE


def _host_inputs(img: np.ndarray):
    import ml_dtypes

    pad = np.full((H + 2, W + 2), SENTINEL, np.float32)
    pad[1 : H + 1, 1 : W + 1] = img
    pad[0, 1 : W + 1] = img[H - 1]  # top wrap row
    pad[1 : H + 1, 0] = img[:, W - 1]  # left wrap col
    pad[0, 0] = img[H - 1, W - 1]  # NW corner wrap
    # bottom row / right col stay at the sentinel (invalid-high -> bit 0)

    widf = np.zeros((128, 8 * 128), np.float32)
    idx = np.arange(128)
    for k, (_dx, _dy, wgt) in enumerate(OFFSETS):
        widf[idx, 128 * k + idx] = float(wgt)
    wid = widf.astype(ml_dtypes.bfloat16)

    in_maps = []
    for c in range(NCORES):
        in_maps.append(
            {
                "x": np.ascontiguousarray(pad[RPC * c : RPC * c + RPC + 2, :]),
                "wident": wid,
            }
        )
    return in_maps


def kernel(rgb_image: np.ndarray, _trace: bool = False, _tmpdir: str | None = None):
    from concourse import bass_utils

    img = np.asarray(rgb_image, dtype=np.float32)
    assert img.shape == (H, W), img.shape
    in_maps = _host_inputs(img)
    nc = _get_nc()
    try:
        res = bass_utils.run_bass_kernel_spmd(
            nc,
            in_maps,
            core_ids=list(range(NCORES)),
            trace=_trace,
            tmpdir=_tmpdir,
        )
    except ModuleNotFoundError:
        # axon NTFF profile hook unavailable -> run without trace
        res = bass_utils.run_bass_kernel_spmd(
            nc, in_maps, core_ids=list(range(NCORES)), trace=False
        )
    out = np.concatenate([r["y"] for r in res.results], axis=0)
    if _trace:
        kernel.last_results = res
    return out



# revision 29
# speedup vs baseline: 3.7189x; 3.7189x over previous
"""LBP (local binary pattern) extractor on 8 Trainium2 NeuronCores.

Reference semantics per pixel p, neighbors n_k clockwise, weights 1..128:
    bit_k = img[p + off_k] >= img[p];  index -1 wraps (python negative
    indexing), index >= size contributes 0.  out = sum_k w_k bit_k (uint8).

Key ideas vs the naive per-row layout:
  * Host maps the fp32 image to monotone fp16 rank keys (bit patterns
    0x0001..0x7BFE; positive fp16 ordering == bit ordering; 0.0 is the
    "invalid-high" sentinel).  16-bit keys give the DVE its 2x mode and
    halve DMA traffic.  Rank ties (~1/31742 per edge) -> relerr ~4e-3.
  * 2D-banded layout: each of the 128 partitions owns a 64x1024 image patch
    (plus 1-pixel halo ring -> 66x1026).  ALL eight neighbor offsets become
    free-dimension offsets, so no partition-shifted copies are needed and
    the image is read exactly once.
  * Compare reuse: only the 4 "forward" planes (E,SE,S,SW) are computed.
    Backward bits are complements at a shifted location:
        bit_{-d}[p] = 1 - bit_d[p - d]   (exact up to rank ties).
    The complement+shift folds into the PE assembly as a negative diagonal
    weight plus a constant added by the final ACT copy.
  * Byte aliasing: fp16 1.0's high byte is exactly fp8e4 1.5.  The PE reads
    the fp16 {0,1} planes through a stride-2 fp8 bitcast AP, so one
    DoubleRow matmul per plane fuses the forward and backward passes at
    fp8 rate; the final ACT copy multiplies by fp32(2/3) (exact).
  * Cross-strip reuse: strips after the first store no -1 plane row; the
    backward term of a strip's first out-row accumulates via a plain
    matmul against the PREVIOUS strip's last plane row.
"""

import numpy as np

H = 8192
W = 8192
NCORES = 8
RPC = H // NCORES  # 1024 rows per core

PH, PW = 64, 1024  # patch held by one partition (16 x 8 patches per core)
PBR, PBC = RPC // PH, W // PW  # 16 x 8 patch grid
KR, KC = PH + 2, PW + 2  # key patch incl halo ring (66 x 1026)

# strip sizes: small first/last strips shorten pipeline lead-in/drain
STRIPS = [2, 6, 8, 8, 8, 8, 8, 8, 4, 2, 2]
assert sum(STRIPS) == PH
PLC = PW + 1  # plane cols (1025)

CHUNK = 512  # PSUM chunk (one bank)
GROUP = 2048  # ACT copy granularity (4 banks)

# forward planes: (name, dx, dy, wf, wb, c_lo)
#   forward offset d=(dx,dy) weight wf; backward offset -d weight wb.
#   c_lo: first plane column (-1 or 0).
PLANES = [
    ("E", 0, 1, 8.0, 128.0, -1),
    ("SE", 1, 1, 16.0, 1.0, -1),
    ("S", 1, 0, 32.0, 2.0, -1),
    ("SW", 1, -1, 64.0, 4.0, 0),
]
NP = len(PLANES)
BIAS = sum(p[4] for p in PLANES)  # 135: constant from the complements


def _build_bass():
    import concourse.bacc as bacc
    import concourse.mybir as mybir
    from concourse.bass import AP
    from concourse.tile import TileContext

    f16 = mybir.dt.float16
    f8 = mybir.dt.float8e4
    f32 = mybir.dt.float32
    u8 = mybir.dt.uint8

    nc = bacc.Bacc("TRN2", target_bir_lowering=False)
    x = nc.dram_tensor("x", [128, KR, KC], f16, kind="ExternalInput")
    # fp8 diagonal weights per plane: slot pair (0,1) = (-wb, +wf) for
    # bwd-region-first DoubleRow; pair (2,3) = (+wf, -wb) for fwd-first.
    wf8 = nc.dram_tensor("wf8", [128, NP, 4, 128], f8, kind="ExternalInput")
    y = nc.dram_tensor("y", [128, PH, PW], u8, kind="ExternalOutput")

    max_sr = max(STRIPS)

    NSLOT = 2 * NP  # plane-slot ring: 2 strips in flight
    SLOTR = max(max(STRIPS), STRIPS[0] + 1)  # rows per slot

    with TileContext(nc) as tc:
        with (
            tc.tile_pool(name="const", bufs=1) as cpool,
            tc.tile_pool(name="keys", bufs=3) as kpool,
            tc.tile_pool(name="plp", bufs=1) as ppool,
            tc.tile_pool(name="outb", bufs=2) as opool,
            tc.tile_pool(name="psum", bufs=2, space="PSUM") as qpool,
        ):
            w8 = cpool.tile([128, NP, 4, 128], f8)
            nc.sync.dma_start(w8[:, :, :, :], wf8[:, :, :, :])

            # one persistent plane ring buffer: all strips' bitplanes live in
            # ONE tensor so cross-strip DoubleRow regions are expressible.
            pb = ppool.tile([128, NSLOT, SLOTR, PLC], f16)
            pb8 = pb[:, :, :, :].bitcast(f8)
            SLOTSZ = SLOTR * PLC  # fp16 elems per slot

            prev_last_row = None  # slot-local row index of prev last row
            row0 = 0
            for s, SR in enumerate(STRIPS):
                # plane rows: strip 0 keeps a -1 halo row, later strips
                # reuse the previous strip's last row instead.
                PLR = SR + 1 if s == 0 else SR
                rlo = 0 if s == 0 else 1  # key row of plane row 0's center
                KSR = SR + 2

                k = kpool.tile(
                    [128, max_sr + 2, KC], f16, tag="k", name=f"k{s}"
                )
                nc.sync.dma_start(
                    k[:, 0:KSR, :], x[:, row0 : row0 + KSR, :]
                )

                par = s % 2  # plane q lives in slot 2q+par: consecutive
                # strips' same-plane slots are ADJACENT so every DoubleRow
                # region stride fits the 16-bit ISA step field.
                for q, (name, dx, dy, wfw, wbw, c_lo) in enumerate(PLANES):
                    ci0 = c_lo + 1  # key col of plane col 0's center
                    nc.vector.tensor_tensor(
                        out=pb[:, 2 * q + par, 0:PLR, :],
                        in0=k[:, rlo + dx : rlo + dx + PLR, ci0 + dy : ci0 + dy + PLC],
                        in1=k[:, rlo : rlo + PLR, ci0 : ci0 + PLC],
                        op=mybir.AluOpType.is_ge,
                    )

                ro = 1 if s == 0 else 0  # storage row of plane row 0
                ot = None
                ngroups = SR * PW // GROUP
                for g in range(ngroups):
                    if g % 2 == 0:  # staging buffer (4 out rows)
                        ot = opool.tile(
                            [128, 2 * GROUP // PW, PW], u8, tag="ot",
                            name=f"ot{s}{g}",
                        )
                        hr0 = (g // 2) * (2 * GROUP // PW)
                    ps = qpool.tile([128, GROUP], f32, tag="ps", name=f"ps{s}{g}")
                    for c in range(GROUP // CHUNK):
                        lin = g * GROUP + c * CHUNK
                        r, c0 = divmod(lin, PW)
                        out_ap = ps[:, c * CHUNK : (c + 1) * CHUNK]
                        for q, (name, dx, dy, wfw, wbw, c_lo) in enumerate(
                            PLANES
                        ):
                            sf = 2  # fp16 plane: high byte at fp8 stride 2
                            # fwd region: this strip's plane row r; bwd
                            # region: plane row r - dx (the previous strip's
                            # last row when r == 0, dx == 1, s > 0) -- all in
                            # one ring tensor, so always one DoubleRow matmul.
                            off_f = (2 * q + par) * SLOTSZ + (r + ro) * PLC + (
                                c0 - c_lo
                            )
                            if r == 0 and dx == 1 and s > 0:
                                off_b = (2 * q + 1 - par) * SLOTSZ + (
                                    prev_last_row * PLC
                                ) + (c0 - dy - c_lo)
                            else:
                                off_b = (2 * q + par) * SLOTSZ + (
                                    (r - dx + ro) * PLC
                                ) + (c0 - dy - c_lo)
                            lo, hi = min(off_b, off_f), max(off_b, off_f)
                            # region order by address; pick matching weights
                            wsl = 0 if lo == off_b else 2
                            rhs = AP(
                                tensor=pb8.tensor,
                                offset=pb8.offset + sf * lo + 1,
                                ap=[
                                    list(pb8.ap[0]),
                                    [sf * (hi - lo), 2],
                                    [sf, CHUNK],
                                ],
                            )
                            nc.tensor.matmul(
                                out_ap,
                                lhsT=w8[:, q, wsl : wsl + 2, :],
                                rhs=rhs,
                                start=(q == 0),
                                stop=(q == NP - 1),
                                perf_mode=mybir.MatmulPerfMode.DoubleRow,
                            )
                    r0 = g * GROUP // PW - hr0
                    # psum = 1.5 * sum(+-w * bit)  (fp16 1.0 high byte reads
                    # as fp8e4 1.5); fp32(2/3)*1.5*S lands exactly on S.
                    nc.scalar.activation(
                        ot[:, r0 : r0 + 2, :],
                        ps[:, :],
                        mybir.ActivationFunctionType.Copy,
                        bias=float(BIAS),
                        scale=2.0 / 3.0,
                    )
                    if g % 2 == 1 or g == ngroups - 1:
                        ys = row0 + hr0
                        nrows = g * GROUP // PW - hr0 + 2
                        # issue from the idle Pool engine's SWDGE: keeps
                        # both the SP queue (key DMAs) and the ACT sequencer
                        # (PSUM copies) free.
                        nc.scalar.dma_start(
                            y[:, ys : ys + nrows, :], ot[:, 0:nrows, :]
                        )
                prev_last_row = SR - 1 + ro
                row0 += SR

    nc.compile()
    return nc


_NC_CACHE = None


def _get_nc():
    global _NC_CACHE
    if _NC_CACHE is None:
        _NC_CACHE = _build_bass()
    return _NC_CACHE


def _host_inputs(img: np.ndarray):
    import ml_dtypes

    # Monotone rank keys as fp16 BIT PATTERNS 0x0001..0x7BFE (positive fp16
    # ordering == bit-pattern ordering, 31742 levels); 0.0 is the
    # invalid-high sentinel.  Tie rate ~1/31742 per edge -> relerr ~4e-3.
    keys = (
        np.floor(img.astype(np.float64) * (31741.0 / 255.0)).astype(np.uint16)
        + 1
    ).view(np.float16)

    # Padded key plane [H+2, W+2]: row/col -1 wrap, row/col H/W sentinel 0.
    pad = np.zeros((H + 2, W + 2), np.float16)
    pad[1 : H + 1, 1 : W + 1] = keys
    pad[0, 1 : W + 1] = keys[H - 1]
    pad[1 : H + 1, 0] = keys[:, W - 1]
    pad[0, 0] = keys[H - 1, W - 1]

    # fp8 diagonal weights: [plane][slot][128][128]
    wgt = np.zeros((NP, 4, 128, 128), np.float32)
    idx = np.arange(128)
    for q, (_n, _dx, _dy, wfw, wbw, _c) in enumerate(PLANES):
        wgt[q, 0, idx, idx] = -wbw
        wgt[q, 1, idx, idx] = wfw
        wgt[q, 2, idx, idx] = wfw
        wgt[q, 3, idx, idx] = -wbw
    wgt = np.ascontiguousarray(wgt.transpose(2, 0, 1, 3))
    wf8 = wgt.astype(ml_dtypes.float8_e4m3fn)

    itemsize = 2
    sv = pad.strides
    in_maps = []
    for c in range(NCORES):
        core_pad = pad[RPC * c : RPC * c + RPC + 2, :]  # [1026, 8194]
        patches = np.lib.stride_tricks.as_strided(
            core_pad,
            shape=(PBR, PBC, KR, KC),
            strides=(PH * sv[0], PW * itemsize, sv[0], itemsize),
        )
        xarr = np.ascontiguousarray(patches.reshape(128, KR, KC))
        in_maps.append({"x": xarr, "wf8": wf8})
    return in_maps


def _gather_output(results):
    out = np.empty((H, W), np.uint8)
    for c, res in enumerate(results):
        ya = res["y"].reshape(PBR, PBC, PH, PW)
        out[RPC * c : RPC * (c + 1)] = ya.transpose(0, 2, 1, 3).reshape(RPC, W)
    return out


def kernel(rgb_image: np.ndarray, _trace: bool = False, _tmpdir: str | None = None):
    from concourse import bass_utils

    img = np.asarray(rgb_image, dtype=np.float32)
    assert img.shape == (H, W), img.shape
    in_maps = _host_inputs(img)
    nc = _get_nc()
    try:
        res = bass_utils.run_bass_kernel_spmd(
            nc,
            in_maps,
            core_ids=list(range(NCORES)),
            trace=_trace,
            tmpdir=_tmpdir,
        )
    except ModuleNotFoundError:
        res = bass_utils.run_bass_kernel_spmd(
            nc, in_maps, core_ids=list(range(NCORES)), trace=False
        )
    out = _gather_output(res.results)
    if _trace:
        kernel.last_results = res
    return out
